# revision 28
# baseline (speedup 1.0000x reference)
"""Trainium2 Bass kernel for nn_Net_41223096107028.

Computes the 4-iteration argaug/attention/masked-MLP loss of reference.py
on 8 NeuronCores, data-parallel over the 2048 (b,t) rows (256 rows/core,
2 partition-tiles of 128).

Per tile and iteration:
  - sliding cross-correlation num[s] = <y_res, window_s(x_res)>: the 255
    shifts are grouped into 16 bands of 16 consecutive shifts.  Each
    band's products are one Pool (gpsimd) tensor_tensor over an
    overlapping-window union rectangle [128, 16, wb] (x's zero pads make
    the rectangle sums exact), reduced to 16 window dots by one DVE
    axis-X tensor_reduce.  Pool streams products while DVE streams
    reduces one band behind — the two engines split the dot work
    (empirically the fastest legal split; Pool cannot run fused
    scalar_tensor_tensor dots and ACT per-shift accumulator reduces
    lose to the group reduce),
  - window norms via two cancellation-free DVE prefix scans of x^2,
  - score num*|num|/ss (monotone in num/sqrt(ss); the positive ||y||
    factor is dropped) via one DVE approx-reciprocal of ss plus
    negate/max/mult; argmax via DVE max_with_indices,
  - per-row window gathers via indirect DMA on a DRAM mirror at
    4-element granularity (4x fewer SWDGE descriptors than per-element),
    with a 4-way is_equal-mask mux on DVE applying the sub-granule
    shift exactly,
  - softmax without max-subtraction (inputs are bounded so exp cannot
    overflow in fp32): one ACT Exp with accumulator, a DVE
    approx-reciprocal of the sum, and a single fused (e*rse)*x_aug
    scalar_tensor_tensor,
  - the 2-layer channel-masked MLP as PE matmuls in transposed layout
    (only the active 256-channel slice is computed),
  - loss via ||y_res_new||^2 (y_ele - y_res = -y_res_new), accumulated
    per-partition with a seeded custom DVE reduce chain and finished on
    the host.

Scheduling: phases are software-pipelined per iteration as
B1(t0) B1(t1) B2(t0) A(t0) B2(t1) A(t1), so each tile's next-iteration
dot work fills the other tile's gather/attention tail latency.  The
x_aug gather is dispatched at the head of B1 so Pool's product stream
for the other tile is never queued behind a descriptor generation that
waits on this tile's argmax.

The x_res/x_attn DRAM mirrors are zero-initialised full-width once and
only their 128-column centers are rewritten per iteration.  The last
iteration skips the x_res update entirely (x_res is dead after it).
"""

import numpy as np

import bass_rust
import concourse.bacc as bacc
import concourse.bass as bass
import concourse.mybir as mybir
import concourse.tile as tile
from concourse import bass_utils
from concourse.masks import make_identity
from concourse.dve_ops import TENSOR_TENSOR_REDUCE

F32 = mybir.dt.float32
U32 = mybir.dt.uint32

B, T, D = 4, 512, 128
HDIM, CDIM = 1024, 256
NI = HDIM // CDIM          # 4 iterations
S = 2 * D - 1              # 255 shifts
PADW = 3 * D - 2           # 382 padded width (SBUF)
MIRW = 384                 # DRAM mirror width (multiple of GO)
GO = 4                     # gather granularity (elements per indirect row)
GROWS = MIRW // GO         # 96 rows per partition in the mirror view
SUPW = D + GO              # 132 gathered elements (33 rows of 4)
NCORES = 8
ROWS = (B * T) // NCORES   # 256 rows per core
NT = ROWS // 128           # 2 partition tiles per core
P = 128
IGNORE_OUT = 10000.0

RECT_MAX = 32              # windows with L <= RECT_MAX go to the rect path
LSPLIT = 90                # L in (RECT_MAX, LSPLIT] -> DVE dots; L > LSPLIT -> Pool

_ALU = mybir.AluOpType
_ACT = mybir.ActivationFunctionType

_NC_CACHE = {}


def _win(s):
    """Valid y-range [d0, d1] of shift s."""
    d0 = max(0, (D - 1) - s)
    d1 = min(D - 1, (2 * D - 2) - s)
    return d0, d1


def _ap(t, offset, pattern):
    return bass_rust.AP(t[:].tensor, offset, pattern)


class _Ctx:
    """Per-build holder for the tiles shared across phases."""


def _setup(tc, cx):
    nc = tc.nc
    cx.nc = nc

    cx.xin = nc.dram_tensor("xin", [ROWS, D], F32, kind="ExternalInput").ap()
    cx.yin = nc.dram_tensor("yin", [ROWS, D], F32, kind="ExternalInput").ap()
    cx.w1t = nc.dram_tensor("w1t", [D, HDIM], F32, kind="ExternalInput").ap()
    cx.w2t = nc.dram_tensor("w2t", [P, HDIM // P, D], F32, kind="ExternalInput").ap()
    cx.b1c = nc.dram_tensor("b1c", [P, HDIM // P], F32, kind="ExternalInput").ap()
    cx.b2c = nc.dram_tensor("b2c", [P, 1], F32, kind="ExternalInput").ap()
    cx.lout = nc.dram_tensor("lsum", [NT, P, NI], F32, kind="ExternalOutput").ap()

    singles, dramp = cx.singles, cx.dramp
    cx.xp = [singles.tile([P, PADW], F32, tag=f"xp{t}", name=f"xp{t}") for t in range(NT)]
    cx.yr = [singles.tile([P, D], F32, tag=f"yr{t}", name=f"yr{t}") for t in range(NT)]
    cx.xap = [singles.tile([P, PADW], F32, tag=f"xap{t}", name=f"xap{t}") for t in range(NT)]
    cx.xpd = [dramp.tile([P, MIRW], F32, tag=f"xpd{t}", name=f"xpd{t}") for t in range(NT)]
    cx.xapd = [dramp.tile([P, MIRW], F32, tag=f"xapd{t}", name=f"xapd{t}") for t in range(NT)]
    cx.w1s = singles.tile([P, HDIM], F32)
    cx.w2s = singles.tile([P, HDIM // P, D], F32)
    cx.b1s = singles.tile([P, HDIM // P], F32)
    cx.b2s = singles.tile([P, 1], F32)
    cx.ident = singles.tile([P, P], F32)
    cx.iota96 = singles.tile([P, 1], U32)    # p*GROWS
    cx.c254 = singles.tile([P, 1], U32)
    cx.zeroD = singles.tile([P, D], F32)
    cx.zpad = singles.tile([P, MIRW], F32)
    cx.lsum = singles.tile([P, NT * NI], F32)

    for t in range(NT):
        nc.gpsimd.memset(cx.xp[t], 0.0)
        nc.gpsimd.memset(cx.xap[t], 0.0)
        nc.sync.dma_start(out=cx.xp[t][:, D - 1 : D - 1 + D],
                          in_=cx.xin[t * P : (t + 1) * P, :])
        nc.sync.dma_start(out=cx.yr[t], in_=cx.yin[t * P : (t + 1) * P, :])
    nc.sync.dma_start(out=cx.w1s, in_=cx.w1t)
    nc.sync.dma_start(out=cx.w2s, in_=cx.w2t)
    nc.sync.dma_start(out=cx.b1s, in_=cx.b1c)
    nc.sync.dma_start(out=cx.b2s, in_=cx.b2c)
    make_identity(nc, cx.ident)
    nc.gpsimd.memset(cx.zeroD, 0.0)
    nc.gpsimd.memset(cx.zpad, 0.0)
    nc.gpsimd.memset(cx.c254, 254)
    nc.gpsimd.iota(cx.iota96, pattern=[[0, 1]], base=0, channel_multiplier=GROWS)
    # zero-init the DRAM mirrors (pads stay valid forever), then write the
    # loaded x_res into the xpd centers for the first iteration's gather.
    for t in range(NT):
        nc.sync.dma_start(out=cx.xpd[t], in_=cx.zpad)
        nc.sync.dma_start(out=cx.xapd[t], in_=cx.zpad)
    for t in range(NT):
        nc.sync.dma_start(out=cx.xpd[t][:, D - 1 : D - 1 + D],
                          in_=cx.xp[t][:, D - 1 : D - 1 + D])


# --- shift-band classification -------------------------------------------
# 16 bands: (side, k) with L in [16k+1, 16(k+1)].  Left side s in [0,127]
# (L = s+1), right side s in [128,254] (L = 255-s; its k=7 band has only
# 15 shifts).  Per-band compute mode:
#   'stt'     - 16 fused mult+accum dots on DVE
#   'rect'    - DVE union-rect product, DVE axis-X group reduce
#   'pooldve' - Pool union-rect product, DVE group reduce
#   'poolact' - Pool union-rect product, per-shift ACT accum reduces
#   'dveact'  - DVE union-rect product, per-shift ACT reduces
# The union rectangle reads xp's zero pads outside each true window, so
# rect sums are exact.
_WIDE_BANDS = False

def _bands():
    if _WIDE_BANDS:
        out = []
        for k in range(4):
            out.append(("L", 2 * k + 1, 32 * k, 32))      # wb = 32(k+1)
        for k in range(4):
            s0 = 255 - 32 * (k + 1)
            cnt = 32
            if k == 3:
                s0, cnt = 128, 31
            out.append(("R", 2 * k + 1, s0, cnt))
        return out
    out = []
    for k in range(8):
        out.append(("L", k, 16 * k, 16))           # s0 = 16k
    for k in range(8):
        s0 = 239 - 16 * k
        cnt = 16
        if k == 7:
            s0, cnt = 128, 15
        out.append(("R", k, s0, cnt))
    return out


_BANDS = _bands()

# mode per (side, k): tuned against the cost model / timeline sim
_MODE = {
    ("L", 0): "rect",    ("R", 0): "rect",
    ("L", 1): "rect",    ("R", 1): "rect",
    ("L", 2): "pooldve", ("R", 2): "stt",
    ("L", 3): "stt",     ("R", 3): "pooldve",
    ("L", 4): "pooldve", ("R", 4): "poolact",
    ("L", 5): "pooldve", ("R", 5): "stt",
    ("L", 6): "stt",     ("R", 6): "stt",
    ("L", 7): "pooldve", ("R", 7): "poolact",
}


def _phase_a(tc, cx, i, t):
    """Dots + score + argmax + x_aug gather launch for (iteration i, tile t)."""
    nc = cx.nc
    work = cx.work
    xp, yr = cx.xp[t], cx.yr[t]

    # window norms: two cancellation-free prefix scans of x^2 (1e-30 seeds
    # guard 0/0 in the score).
    x2m = work.tile([P, D], F32, tag="x2m")
    nc.scalar.activation(x2m, xp[:, D - 1 : D - 1 + D], _ACT.Square)
    ss2 = work.tile([P, S], F32, tag="ss2")
    nc.vector.tensor_tensor_scan(
        out=ss2[:, 0:D], data0=x2m, data1=x2m,
        initial=1e-30, op0=_ALU.add, op1=_ALU.bypass)
    nc.vector.tensor_tensor_scan(
        out=ss2[:, S - 1 : D - 1 : -1],
        data0=x2m[:, D - 1 : 0 : -1], data1=x2m[:, D - 1 : 0 : -1],
        initial=1e-30, op0=_ALU.add, op1=_ALU.bypass)

    num = work.tile([P, S], F32, tag="num")

    # --- window dots, per-band modes.  Emission order matters: Pool/DVE
    # rect products first, then the DVE per-shift dot stream, then the
    # reduces — so no engine head-of-line-blocks on another engine's
    # unfinished products. --------------------------------------------------
    def band_geom(side, k, s0, cnt):
        wb = 16 * (k + 1)
        if side == "L":
            xoff = (D - 1 - (s0 + 15)) + s0  # = 112
            yoff = D - 1 - (s0 + 15)         # 112 - s0
        else:
            xoff = s0
            yoff = 0
        return wb, xoff, yoff

    rects = {}
    prod_d = work.tile([P, D], F32, tag="prod_d")
    act_scr = work.tile([P, D], F32, tag="act_scr")

    for side, k, s0, cnt in _BANDS:          # pass 1: rect products
        mode = _MODE[(side, k)]
        if mode == "stt":
            continue
        wb, xoff, yoff = band_geom(side, k, s0, cnt)
        rect = work.tile([P, cnt, wb], F32, tag=f"rc{side}{k}",
                         name=f"rc{side}{k}")
        rects[(side, k)] = rect
        x_win = _ap(xp, xoff, [[PADW, P], [1, cnt], [1, wb]])
        y_bc = _ap(yr, yoff, [[D, P], [0, cnt], [1, wb]])
        eng = nc.gpsimd if mode in ("pooldve", "poolact") else nc.vector
        eng.tensor_tensor(rect[:], x_win, y_bc, op=_ALU.mult)

    for side, k, s0, cnt in _BANDS:          # pass 2: DVE per-shift dots
        if _MODE[(side, k)] != "stt":
            continue
        for s in range(s0, s0 + cnt):
            d0, d1 = _win(s)
            L = d1 - d0 + 1
            nc.vector.scalar_tensor_tensor(
                out=prod_d[:, :L], in0=yr[:, d0 : d0 + L], scalar=0.0,
                in1=xp[:, s + d0 : s + d0 + L],
                op0=_ALU.bypass, op1=_ALU.mult, accum_out=num[:, s : s + 1])

    for side, k, s0, cnt in _BANDS:          # pass 3: reduces
        mode = _MODE[(side, k)]
        if mode == "stt":
            continue
        wb = 16 * (k + 1)
        rect = rects[(side, k)]
        if mode in ("rect", "pooldve"):
            nc.vector.tensor_reduce(
                num[:, s0 : s0 + cnt], rect[:],
                axis=mybir.AxisListType.X, op=_ALU.add)
        else:  # per-shift ACT accumulator reduces (rows are zero-padded)
            for j in range(cnt):
                nc.scalar.activation(
                    act_scr[:, :wb], rect[:, j, :wb], _ACT.Identity,
                    accum_out=num[:, s0 + j : s0 + j + 1])

    # --- score num*|num|/ss (monotone in num/sqrt(ss); ||y|| dropped) -----
    rec = work.tile([P, S], F32, tag="rec")
    nc.vector.reciprocal_approx_fast(rec, ss2)
    neg = work.tile([P, S], F32, tag="neg")
    _SC = nc.gpsimd if _SCORE_ON_POOL else nc.vector
    _SC.tensor_scalar(out=neg, in0=num, scalar1=-1.0, scalar2=None,
                      op0=_ALU.mult)
    an = work.tile([P, S], F32, tag="an")
    nc.vector.tensor_tensor(an, num, neg, op=_ALU.max)
    nr = work.tile([P, S], F32, tag="nr")
    _SC.tensor_tensor(nr, num, rec, op=_ALU.mult)
    simv = work.tile([P, S], F32, tag="simv")
    _SC.tensor_tensor(simv, nr, an, op=_ALU.mult)
    maxv = work.tile([P, 8], F32, tag="maxv")
    idx8 = work.tile([P, 8], U32, tag="idx8")
    nc.vector.max_with_indices(maxv, idx8, simv)
    idx = idx8[:, 0:1]

    # --- gather offsets: 4-element granule + 4-way masks ------------------
    hi4 = work.tile([P, 1], U32, tag="hi4")
    rem = work.tile([P, 1], U32, tag="rem")
    nc.vector.tensor_scalar(out=hi4, in0=idx, scalar1=2, scalar2=None,
                            op0=_ALU.logical_shift_right)
    nc.vector.tensor_scalar(out=rem, in0=idx, scalar1=3, scalar2=None,
                            op0=_ALU.bitwise_and)
    ma = [work.tile([P, 1], F32, tag=f"ma{k}", name=f"ma{k}") for k in range(GO)]
    cx.masks_a[t] = ma
    for k in range(GO):
        nc.vector.tensor_scalar(out=ma[k], in0=rem, scalar1=k, scalar2=None,
                                op0=_ALU.is_equal)
    cx.hi4[t] = hi4
    cx.idx[t] = idx8
    cx.num[t] = num


def _phase_b1(tc, cx, i, t):
    """Gather launch + mux + attention + reverse-gather for (i, tile t)."""
    nc = cx.nc
    work = cx.work
    yr, xap = cx.yr[t], cx.xap[t]
    idx = cx.idx[t][:, 0:1]
    ma = cx.masks_a[t]
    last = i == NI - 1

    # x_aug gather: dispatched here (not in phase A) so the Pool queue's
    # rect products for the other tile aren't blocked behind a descriptor
    # generation that waits on this tile's argmax.
    offa = work.tile([P, 1], U32, tag="offa")
    nc.gpsimd.tensor_tensor(offa, cx.iota96, cx.hi4[t], op=_ALU.add)
    sup = work.tile([P, SUPW], F32, tag="sup_a", name="sup_a")
    nc.gpsimd.indirect_dma_start(
        out=sup, out_offset=None,
        in_=cx.xpd[t][:].rearrange("p (w o) -> (p w) o", o=GO),
        in_offset=bass.IndirectOffsetOnAxis(ap=offa, axis=0))

    if not last:
        # reverse-gather offsets/masks depend only on idx: compute first so
        # the Pool-side descriptor generation is ready as soon as the
        # attention mirror lands.
        u = work.tile([P, 1], U32, tag="u")
        nc.vector.tensor_tensor(u, cx.c254, idx, op=_ALU.subtract)
        hi_e = work.tile([P, 1], U32, tag="hi_e")
        rem_e = work.tile([P, 1], U32, tag="rem_e")
        nc.vector.tensor_scalar(out=hi_e, in0=u, scalar1=2, scalar2=None,
                                op0=_ALU.logical_shift_right)
        nc.vector.tensor_scalar(out=rem_e, in0=u, scalar1=3, scalar2=None,
                                op0=_ALU.bitwise_and)
        me = [work.tile([P, 1], F32, tag=f"me{k}", name=f"me{k}") for k in range(GO)]
        for k in range(GO):
            nc.vector.tensor_scalar(out=me[k], in0=rem_e, scalar1=k,
                                    scalar2=None, op0=_ALU.is_equal)
        offe = work.tile([P, 1], U32, tag="offe")
        nc.gpsimd.tensor_tensor(offe, cx.iota96, hi_e, op=_ALU.add)
        cx.masks_e[t] = me

    # x_aug via 4-way mask mux of the gathered superset
    xaug = work.tile([P, D], F32, tag="xaug")
    nc.vector.tensor_scalar(out=xaug, in0=sup[:, 0:D], scalar1=ma[0],
                            scalar2=None, op0=_ALU.mult)
    for k in range(1, GO):
        nc.vector.scalar_tensor_tensor(
            out=xaug, in0=sup[:, k : k + D], scalar=ma[k], in1=xaug,
            op0=_ALU.mult, op1=_ALU.add)

    # attention: x_attn = x_aug * e / sum(e), e = exp(x_aug*y) (inputs are
    # small enough that the max-subtraction is unnecessary in fp32)
    tmul = work.tile([P, D], F32, tag="tmul")
    nc.vector.tensor_tensor(tmul, xaug, yr, op=_ALU.mult)
    e1 = work.tile([P, D], F32, tag="e1")
    se = work.tile([P, 1], F32, tag="se")
    nc.scalar.activation(e1, tmul, _ACT.Exp, accum_out=se)
    rse = work.tile([P, 1], F32, tag="rse")
    nc.vector.reciprocal_approx_fast(rse, se)
    # x_attn written straight into the padded reverse-shift buffer
    nc.vector.scalar_tensor_tensor(
        out=xap[:, D - 1 : D - 1 + D], in0=e1, scalar=rse, in1=xaug,
        op0=_ALU.mult, op1=_ALU.mult)

    if not last:
        # mirror the attention center, then reverse-shift gather
        nc.sync.dma_start(out=cx.xapd[t][:, D - 1 : D - 1 + D],
                          in_=xap[:, D - 1 : D - 1 + D])
        sup_e = work.tile([P, SUPW], F32, tag="sup_e")
        nc.gpsimd.indirect_dma_start(
            out=sup_e, out_offset=None,
            in_=cx.xapd[t][:].rearrange("p (w o) -> (p w) o", o=GO),
            in_offset=bass.IndirectOffsetOnAxis(ap=offe, axis=0))
        cx.sup_e[t] = sup_e
    cx.xaug[t] = xaug


def _phase_b2(tc, cx, i, t):
    """Reverse shift apply + MLP + loss for (iteration i, tile t)."""
    nc = cx.nc
    work, psum = cx.work, cx.psum
    xp, yr, xap = cx.xp[t], cx.yr[t], cx.xap[t]
    last = i == NI - 1

    if not last:
        sup_e, me = cx.sup_e[t], cx.masks_e[t]
        # x_ele accumulation + x_res update
        xele = work.tile([P, D], F32, tag="xele")
        nc.vector.tensor_scalar(out=xele, in0=sup_e[:, 0:D], scalar1=me[0],
                                scalar2=None, op0=_ALU.mult)
        for k in range(1, GO):
            nc.vector.scalar_tensor_tensor(
                out=xele, in0=sup_e[:, k : k + D], scalar=me[k], in1=xele,
                op0=_ALU.mult, op1=_ALU.add)
        nc.vector.tensor_tensor(
            xp[:, D - 1 : D - 1 + D], xp[:, D - 1 : D - 1 + D], xele,
            op=_ALU.subtract)
        nc.sync.dma_start(out=cx.xpd[t][:, D - 1 : D - 1 + D],
                          in_=xp[:, D - 1 : D - 1 + D])

    # --- masked 2-layer MLP in transposed layout --------------------------
    hblks = (2 * i, 2 * i + 1)
    xT_ps = psum.tile([P, P], F32, tag="xT_ps")
    nc.tensor.transpose(out=xT_ps, in_=xap[:, D - 1 : D - 1 + D],
                        identity=cx.ident)
    xT = work.tile([P, P], F32, tag="xT")
    nc.scalar.activation(xT, xT_ps, _ACT.Copy)
    y_ps = psum.tile([P, P], F32, tag="y_ps")
    for j, hb in enumerate(hblks):
        h_ps = psum.tile([P, P], F32, tag=f"h_ps{j}")
        nc.tensor.matmul(h_ps, lhsT=cx.w1s[:, hb * P : (hb + 1) * P],
                         rhs=xT, start=True, stop=True)
        hT = work.tile([P, P], F32, tag=f"hT{j}")
        nc.scalar.activation(hT, h_ps, _ACT.Identity,
                             bias=cx.b1s[:, hb : hb + 1])
        nc.tensor.matmul(y_ps, lhsT=cx.w2s[:, hb, :], rhs=hT,
                         start=(j == 0), stop=(j == 1))
    yT = work.tile([P, P], F32, tag="yT")
    nc.scalar.activation(yT, y_ps, _ACT.Identity, bias=cx.b2s[:, 0:1])
    ye_ps = psum.tile([P, P], F32, tag="ye_ps")
    nc.tensor.transpose(out=ye_ps, in_=yT, identity=cx.ident)

    # --- residual update + loss: sq = (y_ele-y_res)^2 = y_new^2 -----------
    nc.vector.tensor_tensor(yr, yr, ye_ps, op=_ALU.subtract)
    slot = t * NI + i
    prev = 0.0 if i == 0 else cx.lsum[:, slot - 1 : slot]
    prod2 = work.tile([P, D], F32, tag="prod2")
    nc.vector._custom_dve(
        TENSOR_TENSOR_REDUCE,
        out=prod2, in0=yr, in1=yr, s0=prev, s1=1.0,
        accum_out=cx.lsum[:, slot : slot + 1])


def _body(tc):
    cx = _Ctx()
    with (
        tc.tile_pool(name="singles", bufs=1) as singles,
        tc.tile_pool(name="dramp", bufs=1, space="DRAM") as dramp,
        tc.tile_pool(name="work", bufs=2) as work,
        tc.tile_pool(name="psum", bufs=1, space="PSUM") as psum,
    ):
        cx.singles, cx.dramp, cx.work, cx.psum = singles, dramp, work, psum
        cx.idx = [None] * NT
        cx.num = [None] * NT
        cx.hi4 = [None] * NT
        cx.masks_a = [None] * NT
        cx.sup_e = [None] * NT
        cx.masks_e = [None] * NT
        cx.xaug = [None] * NT
        _setup(tc, cx)
        nc = cx.nc

        for t in range(NT):
            _phase_a(tc, cx, 0, t)
        for i in range(NI):
            for t in range(NT):
                _phase_b1(tc, cx, i, t)
            for t in range(NT):
                _phase_b2(tc, cx, i, t)
                if i + 1 < NI:
                    _phase_a(tc, cx, i + 1, t)

        for t in range(NT):
            nc.sync.dma_start(out=cx.lout[t],
                              in_=cx.lsum[:, t * NI : (t + 1) * NI])


def build_nc():
    if "nc" in _NC_CACHE:
        return _NC_CACHE["nc"]
    nc = bacc.Bacc("TRN2", target_bir_lowering=False, debug=False,
                   enable_asserts=True, num_devices=NCORES)
    with tile.TileContext(nc) as tc:
        _body(tc)
    nc.compile()
    _NC_CACHE["nc"] = nc
    return nc


def make_in_maps(x, y, w1, b1, w2, b2):
    x = np.ascontiguousarray(np.asarray(x, np.float32)).reshape(B * T, D)
    y = np.ascontiguousarray(np.asarray(y, np.float32)).reshape(B * T, D)
    w1 = np.asarray(w1, np.float32)
    b1 = np.asarray(b1, np.float32)
    w2 = np.asarray(w2, np.float32)
    b2 = np.asarray(b2, np.float32)
    w1t = np.ascontiguousarray(w1.T)                      # (128, 1024)
    w2t = np.ascontiguousarray(                            # (128, 8, 128)
        w2.T.reshape(HDIM // P, P, D).transpose(1, 0, 2))
    b1c = np.ascontiguousarray(b1.reshape(HDIM // P, P).T)  # (128, 8)
    b2c = np.ascontiguousarray(b2.reshape(D, 1))             # (128, 1)
    maps = []
    for c in range(NCORES):
        maps.append({
            "xin": np.ascontiguousarray(x[c * ROWS : (c + 1) * ROWS]),
            "yin": np.ascontiguousarray(y[c * ROWS : (c + 1) * ROWS]),
            "w1t": w1t, "w2t": w2t, "b1c": b1c, "b2c": b2c,
        })
    return maps


def finalize(lsums, y):
    """lsums: list of per-core (NT, P, NI) partial sums of squares."""
    denom = np.float64((np.asarray(y) != IGNORE_OUT).sum())
    total = np.float64(0.0)
    for ls in lsums:
        # slot NI-1 of each (t) chain holds that tile's total over iterations
        total += np.float64(ls[:, :, NI - 1].sum(dtype=np.float64))
    return np.float32(total / denom / NI)


def kernel(x, y, w1, b1, w2, b2):
    nc = build_nc()
    in_maps = make_in_maps(x, y, w1, b1, w2, b2)
    res = bass_utils.run_bass_kernel_spmd(nc, in_maps, core_ids=list(range(NCORES)))
    lsums = [res.results[c]["lsum"] for c in range(NCORES)]
    return finalize(lsums, y)
